# revision 7
# baseline (speedup 1.0000x reference)
"""Trainium2 Bass kernel for MultiHeadLinearAttentionWithCache.

Reference computation (per batch row b, head h):
    v,q,k = x@W* + b*                  (E=1024 -> H=16 heads x D=64)
    q,k   = elu(.)+1
    s_new = s + k (outer) v            (state update, the big tensor)
    z_new = z + k
    out   = ((q . s_new) / (q . z_new + 1e-6)) @ Wo + bo

Sharding: pure data parallel over the batch dim across 8 NeuronCores.

Per-core design (512 rows):
  - projections: PE matmuls, lhsT = xT chunks (host-pretransposed, bf16),
    rhs = W chunks (host-converted bf16); biases folded in as a K=1 matmul
    with a ones-row so they ride the PSUM accumulation.
  - elu(x)+1 = exp(min(x,0)) + relu(x)   (DVE min/max + ACT Exp)
  - state update streams s in natural layout [128b, 4096] per (b-tile, head):
      DVE builds k (outer) v via step-0 broadcast APs,
      the "+" runs on the TensorEngine as two identity matmuls accumulating
      into PSUM (saves ~270us of DVE adds), ScalarE evacuates PSUM->SBUF
      in place, DMA streams s_new out.
  - normalized read: GPSIMD does u = q_bcast * s_new, DVE group-reduces over
    d, then values = qs * reciprocal(qz + 1e-6) and the out-projection.
"""

import numpy as np
import ml_dtypes

B, E, H, D = 4096, 1024, 16, 64
N_CORES = 8
BL = B // N_CORES  # rows per core

_BUILD_CACHE: dict = {}


def build_module(bl: int = BL, c_engine: str = "gpsimd"):
    """Build the per-core Bass module (bl = batch rows on this core)."""
    from contextlib import ExitStack

    import concourse.bass as bass
    import concourse.tile as tile
    from concourse import bacc, mybir

    f32 = mybir.dt.float32
    bf16 = mybir.dt.bfloat16
    FT = mybir.ActivationFunctionType
    NT = bl // 128  # number of 128-row b-tiles

    nc = bacc.Bacc("TRN2", target_bir_lowering=False, debug=False)

    xT_d = nc.dram_tensor("xT", (E, bl), bf16, kind="ExternalInput").ap()
    s_d = nc.dram_tensor("s", (bl, H, D, D), f32, kind="ExternalInput").ap()
    z_d = nc.dram_tensor("z", (bl, H, D), f32, kind="ExternalInput").ap()
    W_d = {
        n: nc.dram_tensor(n, (E, E), bf16, kind="ExternalInput").ap()
        for n in ("Wv", "Wq", "Wk", "Wo")
    }
    b_d = {
        n: nc.dram_tensor(n, (E,), bf16, kind="ExternalInput").ap()
        for n in ("bv", "bq", "bk", "bo")
    }
    id_d = nc.dram_tensor("ident", (128, 128), f32, kind="ExternalInput").ap()
    out_d = nc.dram_tensor("out", (bl, E), f32, kind="ExternalOutput").ap()
    sn_d = nc.dram_tensor("s_new", (bl, H, D, D), f32, kind="ExternalOutput").ap()
    zn_d = nc.dram_tensor("z_new", (bl, H, D), f32, kind="ExternalOutput").ap()

    with tile.TileContext(nc) as tc, ExitStack() as ctx:
        const = ctx.enter_context(tc.tile_pool(name="const", bufs=1))
        xtp = ctx.enter_context(tc.tile_pool(name="xtp", bufs=2))
        qkvp = ctx.enter_context(tc.tile_pool(name="qkvp", bufs=2))
        shp = ctx.enter_context(tc.tile_pool(name="shp", bufs=2))
        uhp = ctx.enter_context(tc.tile_pool(name="uhp", bufs=1))
        tmpp = ctx.enter_context(tc.tile_pool(name="tmpp", bufs=2))
        elup = ctx.enter_context(tc.tile_pool(name="elup", bufs=2))
        smallp = ctx.enter_context(tc.tile_pool(name="smallp", bufs=2))
        outp = ctx.enter_context(tc.tile_pool(name="outp", bufs=1))
        psum_s = ctx.enter_context(
            tc.tile_pool(name="psum_s", bufs=3, space="PSUM")
        )
        psum_pj = ctx.enter_context(
            tc.tile_pool(name="psum_pj", bufs=4, space="PSUM")
        )

        # ---- constants: weights, biases, identity, ones-row ----
        W_sb = {}
        for n in ("Wv", "Wq", "Wk", "Wo"):
            w = const.tile([128, 8, E], bf16, tag=f"W_{n}")
            nc.sync.dma_start(
                w[:], W_d[n].rearrange("(kc p) n -> p kc n", p=128)
            )
            W_sb[n] = w
        bias_sb = const.tile([1, 4 * E], bf16, tag="bias")
        for j, n in enumerate(("bv", "bq", "bk", "bo")):
            nc.sync.dma_start(
                bias_sb[0:1, j * E : (j + 1) * E], b_d[n].unsqueeze(0)
            )
        ident = const.tile([128, 128], f32, tag="ident")
        nc.sync.dma_start(ident[:], id_d[:])
        ones = const.tile([1, 128], bf16, tag="ones")
        nc.gpsimd.memset(ones[:], 1.0)

        BIAS_IDX = {"Wv": 0, "Wq": 1, "Wk": 2, "Wo": 3}

        def project(dst_cb, wname, xTt):
            """dst_cb(nchunk, psum_tile): consume one [128,512] output chunk."""
            for nch in range(2):
                ps = psum_pj.tile([128, 512], f32, tag="pj")
                for kc in range(8):
                    nc.tensor.matmul(
                        ps[:],
                        xTt[:, kc, :],
                        W_sb[wname][:, kc, nch * 512 : (nch + 1) * 512],
                        start=(kc == 0),
                        stop=False,
                    )
                j = BIAS_IDX[wname]
                off = j * E + nch * 512
                nc.tensor.matmul(
                    ps[:],
                    ones[:],
                    bias_sb[0:1, off : off + 512],
                    start=False,
                    stop=True,
                )
                dst_cb(nch, ps)

        for t in range(NT):
            rows = slice(t * 128, (t + 1) * 128)

            # ---- load xT chunks for this b-tile ----
            xTt = xtp.tile([128, 8, 128], bf16, tag="xT")
            nc.sync.dma_start(
                xTt[:], xT_d[:, rows].rearrange("(kc p) b -> p kc b", p=128)
            )

            # ---- projections ----
            v_nat = qkvp.tile([128, E], f32, tag="v")
            q_nat = qkvp.tile([128, E], f32, tag="q")
            k_nat = qkvp.tile([128, E], f32, tag="k")

            def evac_v(nch, ps):
                nc.scalar.copy(v_nat[:, nch * 512 : (nch + 1) * 512], ps[:])

            project(evac_v, "Wv", xTt)

            def make_elu_evac(dst):
                def evac(nch, ps):
                    sl = slice(nch * 512, (nch + 1) * 512)
                    mn = elup.tile([128, 512], f32, tag="mn")
                    mx = elup.tile([128, 512], f32, tag="mx")
                    nc.vector.tensor_scalar_min(mn[:], ps[:], 0.0)
                    nc.vector.tensor_scalar_max(mx[:], ps[:], 0.0)
                    nc.scalar.activation(dst[:, sl], mn[:], FT.Exp)
                    nc.vector.tensor_add(dst[:, sl], dst[:, sl], mx[:])

                return evac

            project(make_elu_evac(q_nat), "Wq", xTt)
            project(make_elu_evac(k_nat), "Wk", xTt)

            # ---- z_new = z + k ; qz = sum_d q*z_new ----
            zt = smallp.tile([128, H * D], f32, tag="z")
            nc.sync.dma_start(zt[:], z_d[rows])
            nc.vector.tensor_add(zt[:], zt[:], k_nat[:])
            nc.sync.dma_start(zn_d[rows], zt[:])
            vals = smallp.tile([128, E], f32, tag="vals")
            nc.vector.tensor_mul(vals[:], q_nat[:], zt[:])
            qz = smallp.tile([128, H], f32, tag="qz")
            nc.vector.reduce_sum(
                qz[:],
                vals[:].rearrange("p (h d) -> p h d", h=H),
                axis=mybir.AxisListType.X,
            )
            nc.vector.tensor_scalar_add(qz[:], qz[:], 1e-6)
            qzr = smallp.tile([128, H], f32, tag="qzr")
            nc.vector.reciprocal(qzr[:], qz[:])

            qs_all = smallp.tile([128, E], f32, tag="qs")

            # ---- state stream per head ----
            for h in range(H):
                sh = shp.tile([128, H * D * D // H], f32, tag="sh")  # [128, 4096]
                nc.sync.dma_start(
                    sh[:].rearrange("p (d m) -> p d m", d=D), s_d[rows, h]
                )
                col = slice(h * D, (h + 1) * D)
                for sl in range(8):
                    fsl = slice(sl * 512, (sl + 1) * 512)
                    tmp = tmpp.tile([128, 512], f32, tag="tmp")
                    k_b = (
                        k_nat[:, h * D + sl * 8 : h * D + sl * 8 + 8]
                        .unsqueeze(2)
                        .broadcast_to([128, 8, D])
                    )
                    v_b = v_nat[:, col].unsqueeze(1).broadcast_to([128, 8, D])
                    nc.vector.tensor_mul(
                        tmp[:].rearrange("p (d m) -> p d m", d=8), k_b, v_b
                    )
                    ps = psum_s.tile([128, 512], f32, tag="ps")
                    nc.tensor.matmul(
                        ps[:], ident[:], sh[:, fsl], start=True, stop=False
                    )
                    nc.tensor.matmul(
                        ps[:], ident[:], tmp[:], start=False, stop=True
                    )
                    nc.scalar.copy(sh[:, fsl], ps[:])

                # u = q_bcast * s_new ; qs_h = sum_d u
                u = uhp.tile([128, D * D], f32, tag="u")
                q_b = q_nat[:, col].unsqueeze(2).broadcast_to([128, D, D])
                s_v = sh[:].rearrange("p (d m) -> p d m", d=D)
                eng = nc.gpsimd if c_engine == "gpsimd" else nc.vector
                eng.tensor_mul(u[:].rearrange("p (d m) -> p d m", d=D), q_b, s_v)
                nc.vector.reduce_sum(
                    qs_all[:, col],
                    u[:].rearrange("p (d m) -> p m d", d=D),
                    axis=mybir.AxisListType.X,
                )
                nc.sync.dma_start(
                    sn_d[rows, h], sh[:].rearrange("p (d m) -> p d m", d=D)
                )

            # ---- values = qs / (qz + eps) ----
            qzr_b = qzr[:].unsqueeze(2).broadcast_to([128, H, D])
            nc.vector.tensor_mul(
                vals[:].rearrange("p (h m) -> p h m", h=H),
                qs_all[:].rearrange("p (h m) -> p h m", h=H),
                qzr_b,
            )

            # ---- transpose values, out-projection ----
            valT = xtp.tile([128, 8, 128], bf16, tag="valT")
            for kc in range(8):
                pst = psum_pj.tile([128, 128], f32, tag="pj")
                nc.tensor.transpose(
                    pst[:], vals[:, kc * 128 : (kc + 1) * 128], ident[:]
                )
                nc.scalar.copy(valT[:, kc, :], pst[:])

            outsb = outp.tile([128, E], f32, tag="outsb")

            def evac_o(nch, ps):
                nc.scalar.copy(outsb[:, nch * 512 : (nch + 1) * 512], ps[:])

            project(evac_o, "Wo", valT)
            nc.sync.dma_start(out_d[rows], outsb[:])

    nc.compile()
    return nc


def _get_module(bl: int = BL, c_engine: str = "gpsimd"):
    key = (bl, c_engine)
    if key not in _BUILD_CACHE:
        _BUILD_CACHE[key] = build_module(bl, c_engine)
    return _BUILD_CACHE[key]


def make_in_maps(x, s, z, Wv, bv, Wq, bq, Wk, bk, Wo, bo, n_cores=N_CORES):
    """Host-side shard + preprocess into per-core input maps."""
    bf = ml_dtypes.bfloat16
    xT = np.ascontiguousarray(np.asarray(x, np.float32).T.astype(bf))  # [E, B]
    shared = {
        "Wv": np.asarray(Wv).astype(bf),
        "Wq": np.asarray(Wq).astype(bf),
        "Wk": np.asarray(Wk).astype(bf),
        "Wo": np.asarray(Wo).astype(bf),
        "bv": np.asarray(bv).astype(bf),
        "bq": np.asarray(bq).astype(bf),
        "bk": np.asarray(bk).astype(bf),
        "bo": np.asarray(bo).astype(bf),
        "ident": np.eye(128, dtype=np.float32),
    }
    bl = x.shape[0] // n_cores
    in_maps = []
    for c in range(n_cores):
        rows = slice(c * bl, (c + 1) * bl)
        in_maps.append(
            {
                "xT": np.ascontiguousarray(xT[:, rows]),
                "s": np.ascontiguousarray(np.asarray(s, np.float32)[rows]),
                "z": np.ascontiguousarray(np.asarray(z, np.float32)[rows]),
                **shared,
            }
        )
    return in_maps


def kernel(x, s, z, Wv, bv, Wq, bq, Wk, bk, Wo, bo):
    from concourse import bass_utils

    nc = _get_module()
    in_maps = make_in_maps(x, s, z, Wv, bv, Wq, bq, Wk, bk, Wo, bo)
    res = bass_utils.run_bass_kernel_spmd(nc, in_maps, list(range(N_CORES)))
    out = np.concatenate([r["out"] for r in res.results], axis=0)
    s_new = np.concatenate([r["s_new"] for r in res.results], axis=0)
    z_new = np.concatenate([r["z_new"] for r in res.results], axis=0)
    return out, s_new, z_new


# revision 10
# speedup vs baseline: 107.9882x; 107.9882x over previous
"""Trainium2 Bass kernel for MultiHeadLinearAttentionWithCache.

Reference computation (per batch row b, head h):
    v,q,k = x@W* + b*                  (E=1024 -> H=16 heads x D=64)
    q,k   = elu(.)+1
    s_new = s + k (outer) v            (state update, the big tensor)
    z_new = z + k
    out   = ((q . s_new) / (q . z_new + 1e-6)) @ Wo + bo

Sharding: pure data parallel over the batch dim across 8 NeuronCores.

Per-core design (512 rows):
  - projections: PE matmuls, lhsT = xT chunks (host-pretransposed, bf16),
    rhs = W chunks (host-converted bf16); biases folded in as a K=1 matmul
    with a ones-row so they ride the PSUM accumulation.
  - elu(x)+1 = exp(min(x,0)) + relu(x)   (DVE min/max + ACT Exp)
  - state update streams s in natural layout [128b, 4096] per (b-tile, head):
      DVE builds k (outer) v via step-0 broadcast APs,
      the "+" runs on the TensorEngine as two identity matmuls accumulating
      into PSUM (saves ~270us of DVE adds), ScalarE evacuates PSUM->SBUF
      in place, DMA streams s_new out.
  - normalized read: GPSIMD does u = q_bcast * s_new, DVE group-reduces over
    d, then values = qs * reciprocal(qz + 1e-6) and the out-projection.
"""

import numpy as np
import ml_dtypes

B, E, H, D = 4096, 1024, 16, 64
N_CORES = 8
BL = B // N_CORES  # rows per core

_BUILD_CACHE: dict = {}


def build_module(bl: int = BL, c_engine: str = "gpsimd", reps: int = 1):
    """Build the per-core Bass module (bl = batch rows on this core).

    reps > 1 wraps the whole body in an on-device For_i loop (the body is
    idempotent) — used only for timing, so the per-call axon transfer cost
    cancels out of a Delta-wall / Delta-reps slope.
    """
    from contextlib import ExitStack

    import concourse.bass as bass
    import concourse.tile as tile
    from concourse import bacc, mybir

    f32 = mybir.dt.float32
    bf16 = mybir.dt.bfloat16
    FT = mybir.ActivationFunctionType
    NT = bl // 128  # number of 128-row b-tiles

    nc = bacc.Bacc("TRN2", target_bir_lowering=False, debug=False)

    xT_d = nc.dram_tensor("xT", (E, bl), bf16, kind="ExternalInput").ap()
    s_d = nc.dram_tensor("s", (bl, H, D, D), f32, kind="ExternalInput").ap()
    z_d = nc.dram_tensor("z", (bl, H, D), f32, kind="ExternalInput").ap()
    W_d = {
        n: nc.dram_tensor(n, (E, E), bf16, kind="ExternalInput").ap()
        for n in ("Wv", "Wq", "Wk", "Wo")
    }
    b_d = {
        n: nc.dram_tensor(n, (E,), bf16, kind="ExternalInput").ap()
        for n in ("bv", "bq", "bk", "bo")
    }
    id_d = nc.dram_tensor("ident", (128, 128), f32, kind="ExternalInput").ap()
    out_d = nc.dram_tensor("out", (bl, E), f32, kind="ExternalOutput").ap()
    sn_d = nc.dram_tensor("s_new", (bl, H, D, D), f32, kind="ExternalOutput").ap()
    zn_d = nc.dram_tensor("z_new", (bl, H, D), f32, kind="ExternalOutput").ap()

    with tile.TileContext(nc) as tc, ExitStack() as ctx:
        const = ctx.enter_context(tc.tile_pool(name="const", bufs=1))
        xtp = ctx.enter_context(tc.tile_pool(name="xtp", bufs=2))
        qkvp = ctx.enter_context(tc.tile_pool(name="qkvp", bufs=2))
        shp = ctx.enter_context(tc.tile_pool(name="shp", bufs=2))
        uhp = ctx.enter_context(tc.tile_pool(name="uhp", bufs=1))
        tmpp = ctx.enter_context(tc.tile_pool(name="tmpp", bufs=2))
        elup = ctx.enter_context(tc.tile_pool(name="elup", bufs=2))
        smallp = ctx.enter_context(tc.tile_pool(name="smallp", bufs=2))
        outp = ctx.enter_context(tc.tile_pool(name="outp", bufs=1))
        psum_s = ctx.enter_context(
            tc.tile_pool(name="psum_s", bufs=3, space="PSUM")
        )
        psum_pj = ctx.enter_context(
            tc.tile_pool(name="psum_pj", bufs=4, space="PSUM")
        )

        # ---- constants: weights, biases, identity, ones-row ----
        W_sb = {}
        for n in ("Wv", "Wq", "Wk", "Wo"):
            w = const.tile([128, 8, E], bf16, tag=f"W_{n}")
            nc.sync.dma_start(
                w[:], W_d[n].rearrange("(kc p) n -> p kc n", p=128)
            )
            W_sb[n] = w
        bias_sb = const.tile([1, 4 * E], bf16, tag="bias")
        for j, n in enumerate(("bv", "bq", "bk", "bo")):
            nc.sync.dma_start(
                bias_sb[0:1, j * E : (j + 1) * E], b_d[n].unsqueeze(0)
            )
        ident = const.tile([128, 128], f32, tag="ident")
        nc.sync.dma_start(ident[:], id_d[:])
        ones = const.tile([1, 128], bf16, tag="ones")
        nc.gpsimd.memset(ones[:], 1.0)

        BIAS_IDX = {"Wv": 0, "Wq": 1, "Wk": 2, "Wo": 3}

        rep_cm = tc.For_i(0, reps, 1, name="rep") if reps > 1 else None
        if rep_cm is not None:
            ctx.enter_context(rep_cm)

        def project(dst_cb, wname, xTt):
            """dst_cb(nchunk, psum_tile): consume one [128,512] output chunk."""
            for nch in range(2):
                ps = psum_pj.tile([128, 512], f32, tag="pj")
                for kc in range(8):
                    nc.tensor.matmul(
                        ps[:],
                        xTt[:, kc, :],
                        W_sb[wname][:, kc, nch * 512 : (nch + 1) * 512],
                        start=(kc == 0),
                        stop=False,
                    )
                j = BIAS_IDX[wname]
                off = j * E + nch * 512
                nc.tensor.matmul(
                    ps[:],
                    ones[:],
                    bias_sb[0:1, off : off + 512],
                    start=False,
                    stop=True,
                )
                dst_cb(nch, ps)

        for t in range(NT):
            rows = slice(t * 128, (t + 1) * 128)

            # ---- load xT chunks for this b-tile ----
            xTt = xtp.tile([128, 8, 128], bf16, tag="xT")
            nc.sync.dma_start(
                xTt[:], xT_d[:, rows].rearrange("(kc p) b -> p kc b", p=128)
            )

            # ---- projections ----
            v_nat = qkvp.tile([128, E], f32, tag="v")
            q_nat = qkvp.tile([128, E], f32, tag="q")
            k_nat = qkvp.tile([128, E], f32, tag="k")

            def evac_v(nch, ps):
                nc.scalar.copy(v_nat[:, nch * 512 : (nch + 1) * 512], ps[:])

            project(evac_v, "Wv", xTt)

            def make_elu_evac(dst):
                def evac(nch, ps):
                    sl = slice(nch * 512, (nch + 1) * 512)
                    mn = elup.tile([128, 512], f32, tag="mn")
                    mx = elup.tile([128, 512], f32, tag="mx")
                    nc.vector.tensor_scalar_min(mn[:], ps[:], 0.0)
                    nc.vector.tensor_scalar_max(mx[:], ps[:], 0.0)
                    nc.scalar.activation(dst[:, sl], mn[:], FT.Exp)
                    nc.vector.tensor_add(dst[:, sl], dst[:, sl], mx[:])

                return evac

            project(make_elu_evac(q_nat), "Wq", xTt)
            project(make_elu_evac(k_nat), "Wk", xTt)

            # ---- z_new = z + k ; qz = sum_d q*z_new ----
            zt = smallp.tile([128, H * D], f32, tag="z")
            nc.sync.dma_start(zt[:], z_d[rows])
            nc.vector.tensor_add(zt[:], zt[:], k_nat[:])
            nc.sync.dma_start(zn_d[rows], zt[:])
            vals = smallp.tile([128, E], f32, tag="vals")
            nc.vector.tensor_mul(vals[:], q_nat[:], zt[:])
            qz = smallp.tile([128, H], f32, tag="qz")
            nc.vector.reduce_sum(
                qz[:],
                vals[:].rearrange("p (h d) -> p h d", h=H),
                axis=mybir.AxisListType.X,
            )
            nc.vector.tensor_scalar_add(qz[:], qz[:], 1e-6)
            qzr = smallp.tile([128, H], f32, tag="qzr")
            nc.vector.reciprocal(qzr[:], qz[:])

            qs_all = smallp.tile([128, E], f32, tag="qs")

            # ---- state stream per head ----
            for h in range(H):
                sh = shp.tile([128, H * D * D // H], f32, tag="sh")  # [128, 4096]
                nc.sync.dma_start(
                    sh[:].rearrange("p (d m) -> p d m", d=D), s_d[rows, h]
                )
                col = slice(h * D, (h + 1) * D)
                for sl in range(8):
                    fsl = slice(sl * 512, (sl + 1) * 512)
                    tmp = tmpp.tile([128, 512], f32, tag="tmp")
                    k_b = (
                        k_nat[:, h * D + sl * 8 : h * D + sl * 8 + 8]
                        .unsqueeze(2)
                        .broadcast_to([128, 8, D])
                    )
                    v_b = v_nat[:, col].unsqueeze(1).broadcast_to([128, 8, D])
                    nc.vector.tensor_mul(
                        tmp[:].rearrange("p (d m) -> p d m", d=8), k_b, v_b
                    )
                    ps = psum_s.tile([128, 512], f32, tag="ps")
                    nc.tensor.matmul(
                        ps[:], ident[:], sh[:, fsl], start=True, stop=False
                    )
                    nc.tensor.matmul(
                        ps[:], ident[:], tmp[:], start=False, stop=True
                    )
                    nc.scalar.copy(sh[:, fsl], ps[:])

                # u = q_bcast * s_new ; qs_h = sum_d u
                u = uhp.tile([128, D * D], f32, tag="u")
                q_b = q_nat[:, col].unsqueeze(2).broadcast_to([128, D, D])
                s_v = sh[:].rearrange("p (d m) -> p d m", d=D)
                eng = nc.gpsimd if c_engine == "gpsimd" else nc.vector
                eng.tensor_mul(u[:].rearrange("p (d m) -> p d m", d=D), q_b, s_v)
                nc.vector.reduce_sum(
                    qs_all[:, col],
                    u[:].rearrange("p (d m) -> p m d", d=D),
                    axis=mybir.AxisListType.X,
                )
                nc.sync.dma_start(
                    sn_d[rows, h], sh[:].rearrange("p (d m) -> p d m", d=D)
                )

            # ---- values = qs / (qz + eps) ----
            qzr_b = qzr[:].unsqueeze(2).broadcast_to([128, H, D])
            nc.vector.tensor_mul(
                vals[:].rearrange("p (h m) -> p h m", h=H),
                qs_all[:].rearrange("p (h m) -> p h m", h=H),
                qzr_b,
            )

            # ---- transpose values, out-projection ----
            valT = xtp.tile([128, 8, 128], bf16, tag="valT")
            for kc in range(8):
                pst = psum_pj.tile([128, 128], f32, tag="pj")
                nc.tensor.transpose(
                    pst[:], vals[:, kc * 128 : (kc + 1) * 128], ident[:]
                )
                nc.scalar.copy(valT[:, kc, :], pst[:])

            outsb = outp.tile([128, E], f32, tag="outsb")

            def evac_o(nch, ps):
                nc.scalar.copy(outsb[:, nch * 512 : (nch + 1) * 512], ps[:])

            project(evac_o, "Wo", valT)
            nc.sync.dma_start(out_d[rows], outsb[:])

    nc.compile()
    return nc


def _get_module(bl: int = BL, c_engine: str = "gpsimd", reps: int = 1):
    key = (bl, c_engine, reps)
    if key not in _BUILD_CACHE:
        _BUILD_CACHE[key] = build_module(bl, c_engine, reps)
    return _BUILD_CACHE[key]


def make_in_maps(x, s, z, Wv, bv, Wq, bq, Wk, bk, Wo, bo, n_cores=N_CORES):
    """Host-side shard + preprocess into per-core input maps."""
    bf = ml_dtypes.bfloat16
    xT = np.ascontiguousarray(np.asarray(x, np.float32).T.astype(bf))  # [E, B]
    shared = {
        "Wv": np.asarray(Wv).astype(bf),
        "Wq": np.asarray(Wq).astype(bf),
        "Wk": np.asarray(Wk).astype(bf),
        "Wo": np.asarray(Wo).astype(bf),
        "bv": np.asarray(bv).astype(bf),
        "bq": np.asarray(bq).astype(bf),
        "bk": np.asarray(bk).astype(bf),
        "bo": np.asarray(bo).astype(bf),
        "ident": np.eye(128, dtype=np.float32),
    }
    bl = x.shape[0] // n_cores
    in_maps = []
    for c in range(n_cores):
        rows = slice(c * bl, (c + 1) * bl)
        in_maps.append(
            {
                "xT": np.ascontiguousarray(xT[:, rows]),
                "s": np.ascontiguousarray(np.asarray(s, np.float32)[rows]),
                "z": np.ascontiguousarray(np.asarray(z, np.float32)[rows]),
                **shared,
            }
        )
    return in_maps


def kernel(x, s, z, Wv, bv, Wq, bq, Wk, bk, Wo, bo):
    from concourse import bass_utils

    nc = _get_module()
    in_maps = make_in_maps(x, s, z, Wv, bv, Wq, bq, Wk, bk, Wo, bo)
    res = bass_utils.run_bass_kernel_spmd(nc, in_maps, list(range(N_CORES)))
    out = np.concatenate([r["out"] for r in res.results], axis=0)
    s_new = np.concatenate([r["s_new"] for r in res.results], axis=0)
    z_new = np.concatenate([r["z_new"] for r in res.results], axis=0)
    return out, s_new, z_new


# revision 12
# speedup vs baseline: 190.2259x; 1.7615x over previous
"""Trainium2 Bass kernel for MultiHeadLinearAttentionWithCache.

Reference computation (per batch row b, head h):
    v,q,k = x@W* + b*                  (E=1024 -> H=16 heads x D=64)
    q,k   = elu(.)+1
    s_new = s + k (outer) v            (state update, the big tensor)
    z_new = z + k
    out   = ((q . s_new) / (q . z_new + 1e-6)) @ Wo + bo

Sharding: pure data parallel over the batch dim across 8 NeuronCores.

Per-core design (512 rows):
  - projections: PE matmuls, lhsT = xT chunks (host-pretransposed, bf16),
    rhs = W chunks (host-converted bf16); biases folded in as a K=1 matmul
    with a ones-row so they ride the PSUM accumulation.
  - elu(x)+1 = exp(min(x,0)) + relu(x)   (DVE min/max + ACT Exp)
  - state update streams s in natural layout [128b, 4096] per (b-tile, head):
      DVE builds k (outer) v via step-0 broadcast APs,
      the "+" runs on the TensorEngine as two identity matmuls accumulating
      into PSUM (saves ~270us of DVE adds), ScalarE evacuates PSUM->SBUF
      in place, DMA streams s_new out.
  - normalized read: GPSIMD does u = q_bcast * s_new, DVE group-reduces over
    d, then values = qs * reciprocal(qz + 1e-6) and the out-projection.
"""

import numpy as np
import ml_dtypes

B, E, H, D = 4096, 1024, 16, 64
N_CORES = 8
BL = B // N_CORES  # rows per core

_BUILD_CACHE: dict = {}


def build_module(bl: int = BL, c_engine: str = "gpsimd", reps: int = 1):
    """Build the per-core Bass module (bl = batch rows on this core).

    reps > 1 wraps the whole body in an on-device For_i loop (the body is
    idempotent) — used only for timing, so the per-call axon transfer cost
    cancels out of a Delta-wall / Delta-reps slope.
    """
    from contextlib import ExitStack

    import concourse.bass as bass
    import concourse.tile as tile
    from concourse import bacc, mybir

    f32 = mybir.dt.float32
    bf16 = mybir.dt.bfloat16
    FT = mybir.ActivationFunctionType
    NT = bl // 128  # number of 128-row b-tiles

    nc = bacc.Bacc("TRN2", target_bir_lowering=False, debug=False)

    xT_d = nc.dram_tensor("xT", (E, bl), bf16, kind="ExternalInput").ap()
    s_d = nc.dram_tensor("s", (bl, H, D, D), f32, kind="ExternalInput").ap()
    z_d = nc.dram_tensor("z", (bl, H, D), f32, kind="ExternalInput").ap()
    W_d = {
        n: nc.dram_tensor(n, (E, E), bf16, kind="ExternalInput").ap()
        for n in ("Wv", "Wq", "Wk", "Wo")
    }
    b_d = {
        n: nc.dram_tensor(n, (E,), bf16, kind="ExternalInput").ap()
        for n in ("bv", "bq", "bk", "bo")
    }
    id_d = nc.dram_tensor("ident", (128, 128), f32, kind="ExternalInput").ap()
    out_d = nc.dram_tensor("out", (bl, E), f32, kind="ExternalOutput").ap()
    sn_d = nc.dram_tensor("s_new", (bl, H, D, D), f32, kind="ExternalOutput").ap()
    zn_d = nc.dram_tensor("z_new", (bl, H, D), f32, kind="ExternalOutput").ap()

    with tile.TileContext(nc) as tc, ExitStack() as ctx:
        const = ctx.enter_context(tc.tile_pool(name="const", bufs=1))
        xtp = ctx.enter_context(tc.tile_pool(name="xtp", bufs=2))
        qkvp = ctx.enter_context(tc.tile_pool(name="qkvp", bufs=2))
        shp = ctx.enter_context(tc.tile_pool(name="shp", bufs=2))
        uhp = ctx.enter_context(tc.tile_pool(name="uhp", bufs=1))
        tmpp = ctx.enter_context(tc.tile_pool(name="tmpp", bufs=2))
        elup = ctx.enter_context(tc.tile_pool(name="elup", bufs=2))
        smallp = ctx.enter_context(tc.tile_pool(name="smallp", bufs=2))
        outp = ctx.enter_context(tc.tile_pool(name="outp", bufs=1))
        psum_s = ctx.enter_context(
            tc.tile_pool(name="psum_s", bufs=3, space="PSUM")
        )
        psum_pj = ctx.enter_context(
            tc.tile_pool(name="psum_pj", bufs=4, space="PSUM")
        )

        # ---- constants: weights, biases, identity, ones-row ----
        W_sb = {}
        for n in ("Wv", "Wq", "Wk", "Wo"):
            w = const.tile([128, 8, E], bf16, tag=f"W_{n}")
            nc.sync.dma_start(
                w[:], W_d[n].rearrange("(kc p) n -> p kc n", p=128)
            )
            W_sb[n] = w
        bias_sb = const.tile([1, 4 * E], bf16, tag="bias")
        for j, n in enumerate(("bv", "bq", "bk", "bo")):
            nc.sync.dma_start(
                bias_sb[0:1, j * E : (j + 1) * E], b_d[n].unsqueeze(0)
            )
        ident = const.tile([128, 128], f32, tag="ident")
        nc.sync.dma_start(ident[:], id_d[:])
        ones = const.tile([1, 128], bf16, tag="ones")
        nc.gpsimd.memset(ones[:], 1.0)

        BIAS_IDX = {"Wv": 0, "Wq": 1, "Wk": 2, "Wo": 3}

        rep_cm = tc.For_i(0, reps, 1, name="rep") if reps > 1 else None
        if rep_cm is not None:
            ctx.enter_context(rep_cm)

        def project(dst_cb, wname, xTt):
            """dst_cb(nchunk, psum_tile): consume one [128,512] output chunk."""
            for nch in range(2):
                ps = psum_pj.tile([128, 512], f32, tag="pj")
                for kc in range(8):
                    nc.tensor.matmul(
                        ps[:],
                        xTt[:, kc, :],
                        W_sb[wname][:, kc, nch * 512 : (nch + 1) * 512],
                        start=(kc == 0),
                        stop=False,
                    )
                j = BIAS_IDX[wname]
                off = j * E + nch * 512
                nc.tensor.matmul(
                    ps[:],
                    ones[:],
                    bias_sb[0:1, off : off + 512],
                    start=False,
                    stop=True,
                )
                dst_cb(nch, ps)

        for t in range(NT):
            rows = slice(t * 128, (t + 1) * 128)

            # ---- load xT chunks for this b-tile ----
            xTt = xtp.tile([128, 8, 128], bf16, tag="xT")
            nc.sync.dma_start(
                xTt[:], xT_d[:, rows].rearrange("(kc p) b -> p kc b", p=128)
            )

            # ---- projections ----
            v_nat = qkvp.tile([128, E], f32, tag="v")
            q_nat = qkvp.tile([128, E], f32, tag="q")
            k_nat = qkvp.tile([128, E], f32, tag="k")

            def evac_v(nch, ps):
                nc.scalar.copy(v_nat[:, nch * 512 : (nch + 1) * 512], ps[:])

            project(evac_v, "Wv", xTt)

            def make_elu_evac(dst):
                def evac(nch, ps):
                    sl = slice(nch * 512, (nch + 1) * 512)
                    mn = elup.tile([128, 512], f32, tag="mn")
                    mx = elup.tile([128, 512], f32, tag="mx")
                    nc.vector.tensor_scalar_min(mn[:], ps[:], 0.0)
                    nc.vector.tensor_scalar_max(mx[:], ps[:], 0.0)
                    nc.scalar.activation(dst[:, sl], mn[:], FT.Exp)
                    nc.vector.tensor_add(dst[:, sl], dst[:, sl], mx[:])

                return evac

            project(make_elu_evac(q_nat), "Wq", xTt)
            project(make_elu_evac(k_nat), "Wk", xTt)

            # ---- z_new = z + k ; qz = sum_d q*z_new ----
            zt = smallp.tile([128, H * D], f32, tag="z")
            nc.sync.dma_start(zt[:], z_d[rows])
            nc.vector.tensor_add(zt[:], zt[:], k_nat[:])
            nc.scalar.dma_start(zn_d[rows], zt[:])
            vals = smallp.tile([128, E], f32, tag="vals")
            nc.vector.tensor_mul(vals[:], q_nat[:], zt[:])
            qz = smallp.tile([128, H], f32, tag="qz")
            nc.vector.reduce_sum(
                qz[:],
                vals[:].rearrange("p (h d) -> p h d", h=H),
                axis=mybir.AxisListType.X,
            )
            nc.vector.tensor_scalar_add(qz[:], qz[:], 1e-6)
            qzr = smallp.tile([128, H], f32, tag="qzr")
            nc.vector.reciprocal(qzr[:], qz[:])

            qs_all = smallp.tile([128, E], f32, tag="qs")

            # ---- state stream per head ----
            for h in range(H):
                sh = shp.tile([128, H * D * D // H], f32, tag="sh")  # [128, 4096]
                nc.sync.dma_start(
                    sh[:].rearrange("p (d m) -> p d m", d=D), s_d[rows, h]
                )
                col = slice(h * D, (h + 1) * D)
                for sl in range(8):
                    fsl = slice(sl * 512, (sl + 1) * 512)
                    tmp = tmpp.tile([128, 512], f32, tag="tmp")
                    k_b = (
                        k_nat[:, h * D + sl * 8 : h * D + sl * 8 + 8]
                        .unsqueeze(2)
                        .broadcast_to([128, 8, D])
                    )
                    v_b = v_nat[:, col].unsqueeze(1).broadcast_to([128, 8, D])
                    nc.vector.tensor_mul(
                        tmp[:].rearrange("p (d m) -> p d m", d=8), k_b, v_b
                    )
                    ps = psum_s.tile([128, 512], f32, tag="ps")
                    nc.tensor.matmul(
                        ps[:], ident[:], sh[:, fsl], start=True, stop=False
                    )
                    nc.tensor.matmul(
                        ps[:], ident[:], tmp[:], start=False, stop=True
                    )
                    nc.scalar.copy(sh[:, fsl], ps[:])

                # u = q_bcast * s_new ; qs_h = sum_d u
                u = uhp.tile([128, D * D], f32, tag="u")
                q_b = q_nat[:, col].unsqueeze(2).broadcast_to([128, D, D])
                s_v = sh[:].rearrange("p (d m) -> p d m", d=D)
                eng = nc.gpsimd if c_engine == "gpsimd" else nc.vector
                eng.tensor_mul(u[:].rearrange("p (d m) -> p d m", d=D), q_b, s_v)
                nc.vector.reduce_sum(
                    qs_all[:, col],
                    u[:].rearrange("p (d m) -> p m d", d=D),
                    axis=mybir.AxisListType.X,
                )
                # separate HWDGE ring (qActDynamicHW) so the s_new write
                # stream runs concurrently with the s read stream
                nc.scalar.dma_start(
                    sn_d[rows, h], sh[:].rearrange("p (d m) -> p d m", d=D)
                )

            # ---- values = qs / (qz + eps) ----
            qzr_b = qzr[:].unsqueeze(2).broadcast_to([128, H, D])
            nc.vector.tensor_mul(
                vals[:].rearrange("p (h m) -> p h m", h=H),
                qs_all[:].rearrange("p (h m) -> p h m", h=H),
                qzr_b,
            )

            # ---- transpose values, out-projection ----
            valT = xtp.tile([128, 8, 128], bf16, tag="valT")
            for kc in range(8):
                pst = psum_pj.tile([128, 128], f32, tag="pj")
                nc.tensor.transpose(
                    pst[:], vals[:, kc * 128 : (kc + 1) * 128], ident[:]
                )
                nc.scalar.copy(valT[:, kc, :], pst[:])

            outsb = outp.tile([128, E], f32, tag="outsb")

            def evac_o(nch, ps):
                nc.scalar.copy(outsb[:, nch * 512 : (nch + 1) * 512], ps[:])

            project(evac_o, "Wo", valT)
            nc.scalar.dma_start(out_d[rows], outsb[:])

    nc.compile()
    return nc


def _get_module(bl: int = BL, c_engine: str = "gpsimd", reps: int = 1):
    key = (bl, c_engine, reps)
    if key not in _BUILD_CACHE:
        _BUILD_CACHE[key] = build_module(bl, c_engine, reps)
    return _BUILD_CACHE[key]


def make_in_maps(x, s, z, Wv, bv, Wq, bq, Wk, bk, Wo, bo, n_cores=N_CORES):
    """Host-side shard + preprocess into per-core input maps."""
    bf = ml_dtypes.bfloat16
    xT = np.ascontiguousarray(np.asarray(x, np.float32).T.astype(bf))  # [E, B]
    shared = {
        "Wv": np.asarray(Wv).astype(bf),
        "Wq": np.asarray(Wq).astype(bf),
        "Wk": np.asarray(Wk).astype(bf),
        "Wo": np.asarray(Wo).astype(bf),
        "bv": np.asarray(bv).astype(bf),
        "bq": np.asarray(bq).astype(bf),
        "bk": np.asarray(bk).astype(bf),
        "bo": np.asarray(bo).astype(bf),
        "ident": np.eye(128, dtype=np.float32),
    }
    bl = x.shape[0] // n_cores
    in_maps = []
    for c in range(n_cores):
        rows = slice(c * bl, (c + 1) * bl)
        in_maps.append(
            {
                "xT": np.ascontiguousarray(xT[:, rows]),
                "s": np.ascontiguousarray(np.asarray(s, np.float32)[rows]),
                "z": np.ascontiguousarray(np.asarray(z, np.float32)[rows]),
                **shared,
            }
        )
    return in_maps


def kernel(x, s, z, Wv, bv, Wq, bq, Wk, bk, Wo, bo):
    from concourse import bass_utils

    nc = _get_module()
    in_maps = make_in_maps(x, s, z, Wv, bv, Wq, bq, Wk, bk, Wo, bo)
    res = bass_utils.run_bass_kernel_spmd(nc, in_maps, list(range(N_CORES)))
    out = np.concatenate([r["out"] for r in res.results], axis=0)
    s_new = np.concatenate([r["s_new"] for r in res.results], axis=0)
    z_new = np.concatenate([r["z_new"] for r in res.results], axis=0)
    return out, s_new, z_new
